# revision 8
# baseline (speedup 1.0000x reference)
"""Causal multi-head attention (GPT-NeoX style) on 8 trn2 NeuronCores.

Full inputs q/k/v: [2, 16, 2048, 128] f32.  Output: [2, 2048, 2048] f32.

Sharding: the 32 (batch, head) pairs are split 4-per-core (head parallel).
Per (b,h) the device kernel computes causal softmax(Q K^T / sqrt(D)) V with
everything transposed ("S^T layout"):

  - host supplies Q^T, K^T as [D=128, S=2048] (d on partitions)
  - MM1: S^T[k_blk, q_chunk] = (K^T blk).T-stationary @ Q^T chunk  -> PSUM
  - ACT: expS = exp(S^T * 1/sqrt(D))                               -> SBUF
  - GPSIMD: multiply diagonal blocks by 0/1 causal masks (from host)
  - DVE: tree-add expS tiles over k -> column partial sums
  - PE: ones-matmul reduces the remaining 128 partitions -> L[1, q]
  - DVE reciprocal + PE broadcast -> 1/L as [128, q]
  - MM2: O^T[d, q] += (V blk)-stationary @ expS blk   (PSUM accumulate)
  - DVE: O^T * (1/L) -> SBUF -> DRAM (host transposes back)

Causality is exploited at 128-block granularity: blocks fully above the
diagonal are never computed (half the matmul/exp work skipped).
"""

import math

import numpy as np

import concourse.bass as bass
import concourse.mybir as mybir
from concourse import bacc
from concourse.tile import TileContext

B, H, S, D = 2, 16, 2048, 128
NCORES = 8
HPC = (B * H) // NCORES  # heads per core = 4
CHUNK = 512              # q-chunk width (1 PSUM bank of f32)
NCHUNK = S // CHUNK      # 4
KB = 128                 # k block
SCALE = 1.0 / math.sqrt(D)
F32 = mybir.dt.float32

_cache = {}


def _build_nc():
    nc = bacc.Bacc()
    qT_d = nc.declare_dram_parameter("qT", [HPC, D, S], F32, isOutput=False)
    kT_d = nc.declare_dram_parameter("kT", [HPC, D, S], F32, isOutput=False)
    v_d = nc.declare_dram_parameter("v", [HPC, S, D], F32, isOutput=False)
    msk_d = nc.declare_dram_parameter("msk", [4, 128, CHUNK], F32, isOutput=False)
    oT_d = nc.declare_dram_parameter("oT", [HPC, D, S], F32, isOutput=True)

    with TileContext(nc) as tc:
        with (
            tc.tile_pool(name="const", bufs=1) as constp,
            tc.tile_pool(name="qkv", bufs=1) as qkvp,
            tc.tile_pool(name="exps", bufs=1) as expp,
            tc.tile_pool(name="work", bufs=1) as workp,
            tc.tile_pool(name="ps", bufs=1, space="PSUM") as psp,
        ):
            masks = constp.tile([128, 4, CHUNK], F32, name="masks")
            nc.sync.dma_start(out=masks, in_=msk_d.rearrange("j p q -> p j q"))
            ones_col = constp.tile([128, 1], F32, name="ones_col")
            nc.vector.memset(ones_col, 1.0)
            ones_row = constp.tile([1, 128], F32, name="ones_row")
            nc.vector.memset(ones_row, 1.0)

            for hd in range(HPC):
                qT = qkvp.tile([D, S], F32, tag="qT", bufs=2, name=f"qT{hd}")
                nc.sync.dma_start(out=qT, in_=qT_d[hd])
                kT = qkvp.tile([D, S], F32, tag="kT", bufs=2, name=f"kT{hd}")
                nc.sync.dma_start(out=kT, in_=kT_d[hd])
                vsb = qkvp.tile([128, S // 128, D], F32, tag="v", bufs=2,
                                name=f"v{hd}")
                nc.sync.dma_start(
                    out=vsb, in_=v_d[hd].rearrange("(t p) d -> p t d", p=128)
                )

                for c in range(NCHUNK):
                    npair = 2 * c + 2  # pairs of 128-k-blocks (incl. 2 diag pairs)
                    qs = c * CHUNK
                    exp_tiles = []
                    for p in range(npair):
                        st = psp.tile([128, 2, CHUNK], F32, tag="sT", bufs=3,
                                      name=f"sT_{hd}_{c}_{p}")
                        et = expp.tile([128, 2, CHUNK], F32, tag="expS", bufs=11,
                                       name=f"expS_{hd}_{c}_{p}")
                        for i in range(2):
                            kb = 2 * p + i
                            nc.tensor.matmul(
                                st[:, i, :],
                                kT[:, kb * KB:(kb + 1) * KB],
                                qT[:, qs:qs + CHUNK],
                                start=True,
                                stop=True,
                            )
                        nc.scalar.activation(
                            et[:, :, :], st[:, :, :],
                            mybir.ActivationFunctionType.Exp,
                            scale=SCALE,
                        )
                        # last two pairs are the diagonal blocks j=0..3
                        if p >= npair - 2:
                            for i in range(2):
                                j = 2 * (p - (npair - 2)) + i
                                nc.gpsimd.tensor_tensor(
                                    out=et[:, i, :], in0=et[:, i, :],
                                    in1=masks[:, j, :], op=mybir.AluOpType.mult,
                                )
                        exp_tiles.append(et)

                    # denominator: tree-add over k blocks (DVE), then reduce
                    # the last 128 partitions with a ones-matmul (PE).
                    accp = workp.tile([128, 2, CHUNK], F32, tag="accp", bufs=2,
                                      name=f"accp_{hd}_{c}")
                    nc.vector.tensor_tensor(
                        out=accp, in0=exp_tiles[0], in1=exp_tiles[1],
                        op=mybir.AluOpType.add,
                    )
                    for p in range(2, npair):
                        nc.vector.tensor_tensor(
                            out=accp, in0=accp, in1=exp_tiles[p],
                            op=mybir.AluOpType.add,
                        )
                    acc = workp.tile([128, CHUNK], F32, tag="acc", bufs=2,
                                     name=f"acc_{hd}_{c}")
                    nc.vector.tensor_tensor(
                        out=acc, in0=accp[:, 0, :], in1=accp[:, 1, :],
                        op=mybir.AluOpType.add,
                    )
                    lb = psp.tile([128, CHUNK], F32, tag="lb", bufs=1,
                                  name=f"lb_{hd}_{c}")
                    nc.tensor.matmul(lb[0:1, :], ones_col, acc,
                                     start=True, stop=True)
                    rl = workp.tile([1, CHUNK], F32, tag="rl", bufs=2,
                                    name=f"rl_{hd}_{c}")
                    nc.vector.reciprocal(rl, lb[0:1, :])
                    nc.tensor.matmul(lb, ones_row, rl, start=True, stop=True)
                    lb_sb = workp.tile([128, CHUNK], F32, tag="lb_sb", bufs=2,
                                       name=f"lbsb_{hd}_{c}")
                    nc.vector.tensor_copy(out=lb_sb, in_=lb)

                    # O^T accumulation over k blocks
                    oT = psp.tile([128, CHUNK], F32, tag="oT", bufs=1,
                                  name=f"oT_{hd}_{c}")
                    nkb = 2 * npair
                    for kb in range(nkb):
                        nc.tensor.matmul(
                            oT,
                            vsb[:, kb, :],
                            exp_tiles[kb // 2][:, kb % 2, :],
                            start=(kb == 0),
                            stop=(kb == nkb - 1),
                        )
                    out_sb = workp.tile([128, CHUNK], F32, tag="out", bufs=2,
                                        name=f"out_{hd}_{c}")
                    nc.vector.tensor_tensor(
                        out=out_sb, in0=oT, in1=lb_sb, op=mybir.AluOpType.mult,
                    )
                    nc.sync.dma_start(out=oT_d[hd][:, qs:qs + CHUNK], in_=out_sb)
    nc.compile()
    return nc


def _host_masks():
    # mask[j][k', q'] = 1 where q' >= k' + 128*j  (valid, keep)
    q = np.arange(CHUNK)[None, :]
    k = np.arange(128)[:, None]
    return np.stack(
        [(q >= k + 128 * j).astype(np.float32) for j in range(4)]
    )


def kernel(query, key, value):
    from concourse.bass_utils import run_bass_kernel_spmd

    if "nc" not in _cache:
        _cache["nc"] = _build_nc()
    nc = _cache["nc"]

    q = np.asarray(query, np.float32).reshape(B * H, S, D)
    k = np.asarray(key, np.float32).reshape(B * H, S, D)
    v = np.asarray(value, np.float32).reshape(B * H, S, D)
    msk = _host_masks()

    in_maps = []
    for c in range(NCORES):
        sl = slice(c * HPC, (c + 1) * HPC)
        in_maps.append({
            "qT": np.ascontiguousarray(q[sl].transpose(0, 2, 1)),
            "kT": np.ascontiguousarray(k[sl].transpose(0, 2, 1)),
            "v": np.ascontiguousarray(v[sl]),
            "msk": msk,
        })

    res = run_bass_kernel_spmd(nc, in_maps, list(range(NCORES))).results

    out = np.empty((B, S, H * D), np.float32)
    for c in range(NCORES):
        oT = res[c]["oT"]  # [HPC, D, S]
        for j in range(HPC):
            g = c * HPC + j
            b, h = g // H, g % H
            out[b, :, h * D:(h + 1) * D] = oT[j].T
    return out


# revision 12
# speedup vs baseline: 12.0871x; 12.0871x over previous
"""Causal multi-head attention (GPT-NeoX style) on 8 trn2 NeuronCores.

Full inputs q/k/v: [2, 16, 2048, 128] f32.  Output: [2, 2048, 2048] f32.

Sharding: the 32 (batch, head) pairs are split 4-per-core (head parallel).
Per (b,h) the device kernel computes causal softmax(Q K^T / sqrt(D)) V with
everything transposed ("S^T layout"):

  - host supplies Q^T, K^T as [D=128, S=2048] (d on partitions)
  - MM1: S^T[k_blk, q_chunk] = (K^T blk).T-stationary @ Q^T chunk  -> PSUM
  - ACT: expS = exp(S^T * 1/sqrt(D))                               -> SBUF
  - GPSIMD: multiply diagonal blocks by 0/1 causal masks (from host)
  - DVE: tree-add expS tiles over k -> column partial sums
  - PE: ones-matmul reduces the remaining 128 partitions -> L[1, q]
  - DVE reciprocal + PE broadcast -> 1/L as [128, q]
  - MM2: O^T[d, q] += (V blk)-stationary @ expS blk   (PSUM accumulate)
  - DVE: O^T * (1/L) -> SBUF -> DRAM (host transposes back)

Causality is exploited at 128-block granularity: blocks fully above the
diagonal are never computed (half the matmul/exp work skipped).
"""

import math

import numpy as np

import concourse.bass as bass
import concourse.mybir as mybir
from concourse import bacc
from concourse.tile import TileContext

B, H, S, D = 2, 16, 2048, 128
NCORES = 8
HPC = (B * H) // NCORES  # heads per core = 4
CHUNK = 512              # q-chunk width (1 PSUM bank of f32)
NCHUNK = S // CHUNK      # 4
KB = 128                 # k block
SCALE = 1.0 / math.sqrt(D)
F32 = mybir.dt.float32

_cache = {}


def _build_nc(reps=1):
    nc = bacc.Bacc()
    qT_d = nc.declare_dram_parameter("qT", [HPC, D, S], F32, isOutput=False)
    kT_d = nc.declare_dram_parameter("kT", [HPC, D, S], F32, isOutput=False)
    v_d = nc.declare_dram_parameter("v", [HPC, S, D], F32, isOutput=False)
    msk_d = nc.declare_dram_parameter("msk", [4, 128, CHUNK], F32, isOutput=False)
    oT_d = nc.declare_dram_parameter("oT", [HPC, D, S], F32, isOutput=True)

    with TileContext(nc) as tc:
        with (
            tc.tile_pool(name="const", bufs=1) as constp,
            tc.tile_pool(name="qkv", bufs=1) as qkvp,
            tc.tile_pool(name="exps", bufs=1) as expp,
            tc.tile_pool(name="work", bufs=1) as workp,
            tc.tile_pool(name="ps", bufs=1, space="PSUM") as psp,
        ):
            masks = constp.tile([128, 4, CHUNK], F32, name="masks")
            nc.sync.dma_start(out=masks, in_=msk_d.rearrange("j p q -> p j q"))
            ones_col = constp.tile([128, 1], F32, name="ones_col")
            nc.vector.memset(ones_col, 1.0)
            ones_row = constp.tile([1, 128], F32, name="ones_row")
            nc.vector.memset(ones_row, 1.0)

            for it in range(reps * HPC):
                hd = it % HPC
                qT = qkvp.tile([D, S], F32, tag="qT", bufs=2, name=f"qT{it}")
                nc.sync.dma_start(out=qT, in_=qT_d[hd])
                kT = qkvp.tile([D, S], F32, tag="kT", bufs=2, name=f"kT{it}")
                nc.sync.dma_start(out=kT, in_=kT_d[hd])
                vsb = qkvp.tile([128, S // 128, D], F32, tag="v", bufs=2,
                                name=f"v{it}")
                nc.sync.dma_start(
                    out=vsb, in_=v_d[hd].rearrange("(t p) d -> p t d", p=128)
                )

                for c in range(NCHUNK):
                    npair = 2 * c + 2  # pairs of 128-k-blocks (incl. 2 diag pairs)
                    qs = c * CHUNK
                    exp_tiles = []
                    for p in range(npair):
                        st = psp.tile([128, 2, CHUNK], F32, tag="sT", bufs=3,
                                      name=f"sT_{it}_{c}_{p}")
                        et = expp.tile([128, 2, CHUNK], F32, tag="expS", bufs=11,
                                       name=f"expS_{it}_{c}_{p}")
                        for i in range(2):
                            kb = 2 * p + i
                            nc.tensor.matmul(
                                st[:, i, :],
                                kT[:, kb * KB:(kb + 1) * KB],
                                qT[:, qs:qs + CHUNK],
                                start=True,
                                stop=True,
                            )
                        nc.scalar.activation(
                            et[:, :, :], st[:, :, :],
                            mybir.ActivationFunctionType.Exp,
                            scale=SCALE,
                        )
                        # last two pairs are the diagonal blocks j=0..3
                        if p >= npair - 2:
                            for i in range(2):
                                j = 2 * (p - (npair - 2)) + i
                                nc.gpsimd.tensor_tensor(
                                    out=et[:, i, :], in0=et[:, i, :],
                                    in1=masks[:, j, :], op=mybir.AluOpType.mult,
                                )
                        exp_tiles.append(et)

                    # denominator: tree-add over k blocks (DVE), then reduce
                    # the last 128 partitions with a ones-matmul (PE).
                    accp = workp.tile([128, 2, CHUNK], F32, tag="accp", bufs=2,
                                      name=f"accp_{it}_{c}")
                    nc.vector.tensor_tensor(
                        out=accp, in0=exp_tiles[0], in1=exp_tiles[1],
                        op=mybir.AluOpType.add,
                    )
                    for p in range(2, npair):
                        nc.vector.tensor_tensor(
                            out=accp, in0=accp, in1=exp_tiles[p],
                            op=mybir.AluOpType.add,
                        )
                    acc = workp.tile([128, CHUNK], F32, tag="acc", bufs=2,
                                     name=f"acc_{it}_{c}")
                    nc.vector.tensor_tensor(
                        out=acc, in0=accp[:, 0, :], in1=accp[:, 1, :],
                        op=mybir.AluOpType.add,
                    )
                    lb = psp.tile([128, CHUNK], F32, tag="lb", bufs=1,
                                  name=f"lb_{it}_{c}")
                    nc.tensor.matmul(lb[0:1, :], ones_col, acc,
                                     start=True, stop=True)
                    rl = workp.tile([1, CHUNK], F32, tag="rl", bufs=2,
                                    name=f"rl_{it}_{c}")
                    nc.vector.reciprocal(rl, lb[0:1, :])
                    nc.tensor.matmul(lb, ones_row, rl, start=True, stop=True)
                    lb_sb = workp.tile([128, CHUNK], F32, tag="lb_sb", bufs=2,
                                       name=f"lbsb_{it}_{c}")
                    nc.vector.tensor_copy(out=lb_sb, in_=lb)

                    # O^T accumulation over k blocks
                    oT = psp.tile([128, CHUNK], F32, tag="oT", bufs=1,
                                  name=f"oT_{it}_{c}")
                    nkb = 2 * npair
                    for kb in range(nkb):
                        nc.tensor.matmul(
                            oT,
                            vsb[:, kb, :],
                            exp_tiles[kb // 2][:, kb % 2, :],
                            start=(kb == 0),
                            stop=(kb == nkb - 1),
                        )
                    out_sb = workp.tile([128, CHUNK], F32, tag="out", bufs=2,
                                        name=f"out_{it}_{c}")
                    nc.vector.tensor_tensor(
                        out=out_sb, in0=oT, in1=lb_sb, op=mybir.AluOpType.mult,
                    )
                    nc.sync.dma_start(out=oT_d[hd][:, qs:qs + CHUNK], in_=out_sb)
    nc.compile()
    return nc


def _host_masks():
    # mask[j][k', q'] = 1 where q' >= k' + 128*j  (valid, keep)
    q = np.arange(CHUNK)[None, :]
    k = np.arange(128)[:, None]
    return np.stack(
        [(q >= k + 128 * j).astype(np.float32) for j in range(4)]
    )


def kernel(query, key, value):
    from concourse.bass_utils import run_bass_kernel_spmd

    if "nc" not in _cache:
        _cache["nc"] = _build_nc()
    nc = _cache["nc"]

    q = np.asarray(query, np.float32).reshape(B * H, S, D)
    k = np.asarray(key, np.float32).reshape(B * H, S, D)
    v = np.asarray(value, np.float32).reshape(B * H, S, D)
    msk = _host_masks()

    in_maps = []
    for c in range(NCORES):
        sl = slice(c * HPC, (c + 1) * HPC)
        in_maps.append({
            "qT": np.ascontiguousarray(q[sl].transpose(0, 2, 1)),
            "kT": np.ascontiguousarray(k[sl].transpose(0, 2, 1)),
            "v": np.ascontiguousarray(v[sl]),
            "msk": msk,
        })

    res = run_bass_kernel_spmd(nc, in_maps, list(range(NCORES))).results

    out = np.empty((B, S, H * D), np.float32)
    for c in range(NCORES):
        oT = res[c]["oT"]  # [HPC, D, S]
        for j in range(HPC):
            g = c * HPC + j
            b, h = g // H, g % H
            out[b, :, h * D:(h + 1) * D] = oT[j].T
    return out


# revision 14
# speedup vs baseline: 22.4167x; 1.8546x over previous
"""Causal multi-head attention (GPT-NeoX style) on 8 trn2 NeuronCores.

Full inputs q/k/v: [2, 16, 2048, 128] f32.  Output: [2, 2048, 2048] f32.

Sharding: the 32 (batch, head) pairs are split 4-per-core (head parallel).
Per (b,h) the device kernel computes causal softmax(Q K^T / sqrt(D)) V with
everything transposed ("S^T layout"):

  - host supplies Q^T, K^T as [D=128, S=2048] (d on partitions)
  - MM1: S^T[k_blk, q_chunk] = (K^T blk).T-stationary @ Q^T chunk  -> PSUM
  - ACT: expS = exp(S^T * 1/sqrt(D))                               -> SBUF
  - GPSIMD: multiply diagonal blocks by 0/1 causal masks (from host)
  - DVE: tree-add expS tiles over k -> column partial sums
  - PE: ones-matmul reduces the remaining 128 partitions -> L[1, q]
  - DVE reciprocal + PE broadcast -> 1/L as [128, q]
  - MM2: O^T[d, q] += (V blk)-stationary @ expS blk   (PSUM accumulate)
  - DVE: O^T * (1/L) -> SBUF -> DRAM (host transposes back)

Causality is exploited at 128-block granularity: blocks fully above the
diagonal are never computed (half the matmul/exp work skipped).
"""

import math

import numpy as np

import concourse.bass as bass
import concourse.mybir as mybir
from concourse import bacc
from concourse.tile import TileContext

B, H, S, D = 2, 16, 2048, 128
NCORES = 8
HPC = (B * H) // NCORES  # heads per core = 4
CHUNK = 512              # q-chunk width (1 PSUM bank of f32)
NCHUNK = S // CHUNK      # 4
KB = 128                 # k block
SCALE = 1.0 / math.sqrt(D)
F32 = mybir.dt.float32

_cache = {}


def _build_nc(reps=1):
    nc = bacc.Bacc()
    qT_d = nc.declare_dram_parameter("qT", [HPC, D, S], F32, isOutput=False)
    kT_d = nc.declare_dram_parameter("kT", [HPC, D, S], F32, isOutput=False)
    v_d = nc.declare_dram_parameter("v", [HPC, S, D], F32, isOutput=False)
    msk_d = nc.declare_dram_parameter("msk", [4, 128, CHUNK], F32, isOutput=False)
    oT_d = nc.declare_dram_parameter("oT", [HPC, D, S], F32, isOutput=True)

    with TileContext(nc) as tc:
        with (
            tc.tile_pool(name="const", bufs=1) as constp,
            tc.tile_pool(name="qkv", bufs=1) as qkvp,
            tc.tile_pool(name="exps", bufs=1) as expp,
            tc.tile_pool(name="work", bufs=1) as workp,
            tc.tile_pool(name="ps", bufs=1, space="PSUM") as psp,
        ):
            masks = constp.tile([128, 4, CHUNK], F32, name="masks")
            nc.sync.dma_start(out=masks, in_=msk_d.rearrange("j p q -> p j q"))
            ones_col = constp.tile([128, 1], F32, name="ones_col")
            nc.vector.memset(ones_col, 1.0)
            ones_row = constp.tile([1, 128], F32, name="ones_row")
            nc.vector.memset(ones_row, 1.0)

            for it in range(reps * HPC):
                hd = it % HPC
                qT = qkvp.tile([D, S], F32, tag="qT", bufs=2, name=f"qT{it}")
                nc.sync.dma_start(out=qT, in_=qT_d[hd])
                kT = qkvp.tile([D, S], F32, tag="kT", bufs=2, name=f"kT{it}")
                nc.sync.dma_start(out=kT, in_=kT_d[hd])
                vsb = qkvp.tile([128, S // 128, D], F32, tag="v", bufs=2,
                                name=f"v{it}")
                nc.sync.dma_start(
                    out=vsb, in_=v_d[hd].rearrange("(t p) d -> p t d", p=128)
                )

                for c in range(NCHUNK):
                    npair = 2 * c + 2  # pairs of 128-k-blocks (incl. 2 diag pairs)
                    qs = c * CHUNK
                    exp_tiles = []
                    for p in range(npair):
                        st = psp.tile([128, 2, CHUNK], F32, tag="sT", bufs=2,
                                      name=f"sT_{it}_{c}_{p}")
                        et = expp.tile([128, 2, CHUNK], F32, tag="expS", bufs=11,
                                       name=f"expS_{it}_{c}_{p}")
                        for i in range(2):
                            kb = 2 * p + i
                            nc.tensor.matmul(
                                st[:, i, :],
                                kT[:, kb * KB:(kb + 1) * KB],
                                qT[:, qs:qs + CHUNK],
                                start=True,
                                stop=True,
                            )
                        nc.scalar.activation(
                            et[:, :, :], st[:, :, :],
                            mybir.ActivationFunctionType.Exp,
                            scale=SCALE,
                        )
                        # last two pairs are the diagonal blocks j=0..3
                        if p >= npair - 2:
                            for i in range(2):
                                j = 2 * (p - (npair - 2)) + i
                                nc.gpsimd.tensor_tensor(
                                    out=et[:, i, :], in0=et[:, i, :],
                                    in1=masks[:, j, :], op=mybir.AluOpType.mult,
                                )
                        exp_tiles.append(et)

                    # denominator: tree-add over k blocks (DVE), then reduce
                    # the last 128 partitions with a ones-matmul (PE).
                    accp = workp.tile([128, 2, CHUNK], F32, tag="accp", bufs=2,
                                      name=f"accp_{it}_{c}")
                    nc.vector.tensor_tensor(
                        out=accp, in0=exp_tiles[0], in1=exp_tiles[1],
                        op=mybir.AluOpType.add,
                    )
                    for p in range(2, npair):
                        nc.vector.tensor_tensor(
                            out=accp, in0=accp, in1=exp_tiles[p],
                            op=mybir.AluOpType.add,
                        )
                    acc = workp.tile([128, CHUNK], F32, tag="acc", bufs=2,
                                     name=f"acc_{it}_{c}")
                    nc.vector.tensor_tensor(
                        out=acc, in0=accp[:, 0, :], in1=accp[:, 1, :],
                        op=mybir.AluOpType.add,
                    )
                    l_ps = psp.tile([1, CHUNK], F32, tag="l", bufs=1,
                                    name=f"l_{it}_{c}")
                    nc.tensor.matmul(l_ps, ones_col, acc,
                                     start=True, stop=True)
                    rl = workp.tile([1, CHUNK], F32, tag="rl", bufs=2,
                                    name=f"rl_{it}_{c}")
                    nc.vector.reciprocal(rl, l_ps)
                    lbb = psp.tile([128, CHUNK], F32, tag="lbb", bufs=1,
                                   name=f"lbb_{it}_{c}")
                    nc.tensor.matmul(lbb, ones_row, rl, start=True, stop=True)
                    lb_sb = workp.tile([128, CHUNK], F32, tag="lb_sb", bufs=2,
                                       name=f"lbsb_{it}_{c}")
                    nc.vector.tensor_copy(out=lb_sb, in_=lbb)

                    # O^T accumulation over k blocks
                    oT = psp.tile([128, CHUNK], F32, tag="oT", bufs=2,
                                  name=f"oT_{it}_{c}")
                    nkb = 2 * npair
                    for kb in range(nkb):
                        nc.tensor.matmul(
                            oT,
                            vsb[:, kb, :],
                            exp_tiles[kb // 2][:, kb % 2, :],
                            start=(kb == 0),
                            stop=(kb == nkb - 1),
                        )
                    out_sb = workp.tile([128, CHUNK], F32, tag="out", bufs=2,
                                        name=f"out_{it}_{c}")
                    nc.vector.tensor_tensor(
                        out=out_sb, in0=oT, in1=lb_sb, op=mybir.AluOpType.mult,
                    )
                    nc.sync.dma_start(out=oT_d[hd][:, qs:qs + CHUNK], in_=out_sb)
    nc.compile()
    return nc


def _host_masks():
    # mask[j][k', q'] = 1 where q' >= k' + 128*j  (valid, keep)
    q = np.arange(CHUNK)[None, :]
    k = np.arange(128)[:, None]
    return np.stack(
        [(q >= k + 128 * j).astype(np.float32) for j in range(4)]
    )


def kernel(query, key, value):
    from concourse.bass_utils import run_bass_kernel_spmd

    if "nc" not in _cache:
        _cache["nc"] = _build_nc()
    nc = _cache["nc"]

    q = np.asarray(query, np.float32).reshape(B * H, S, D)
    k = np.asarray(key, np.float32).reshape(B * H, S, D)
    v = np.asarray(value, np.float32).reshape(B * H, S, D)
    msk = _host_masks()

    in_maps = []
    for c in range(NCORES):
        sl = slice(c * HPC, (c + 1) * HPC)
        in_maps.append({
            "qT": np.ascontiguousarray(q[sl].transpose(0, 2, 1)),
            "kT": np.ascontiguousarray(k[sl].transpose(0, 2, 1)),
            "v": np.ascontiguousarray(v[sl]),
            "msk": msk,
        })

    res = run_bass_kernel_spmd(nc, in_maps, list(range(NCORES))).results

    out = np.empty((B, S, H * D), np.float32)
    for c in range(NCORES):
        oT = res[c]["oT"]  # [HPC, D, S]
        for j in range(HPC):
            g = c * HPC + j
            b, h = g // H, g % H
            out[b, :, h * D:(h + 1) * D] = oT[j].T
    return out


# revision 16
# speedup vs baseline: 24.1014x; 1.0752x over previous
"""Causal multi-head attention (GPT-NeoX style) on 8 trn2 NeuronCores.

Full inputs q/k/v: [2, 16, 2048, 128] f32.  Output: [2, 2048, 2048] f32.

Sharding: the 32 (batch, head) pairs are split 4-per-core (head parallel).
Per (b,h) the device kernel computes causal softmax(Q K^T / sqrt(D)) V with
everything transposed ("S^T layout"):

  - host supplies Q^T, K^T as [D=128, S=2048] (d on partitions)
  - MM1: S^T[k_blk, q_chunk] = (K^T blk).T-stationary @ Q^T chunk  -> PSUM
  - ACT: expS = exp(S^T * 1/sqrt(D))                               -> SBUF
  - GPSIMD: multiply diagonal blocks by 0/1 causal masks (from host)
  - DVE: tree-add expS tiles over k -> column partial sums
  - PE: ones-matmul reduces the remaining 128 partitions -> L[1, q]
  - DVE reciprocal + PE broadcast -> 1/L as [128, q]
  - MM2: O^T[d, q] += (V blk)-stationary @ expS blk   (PSUM accumulate)
  - DVE: O^T * (1/L) -> SBUF -> DRAM (host transposes back)

Causality is exploited at 128-block granularity: blocks fully above the
diagonal are never computed (half the matmul/exp work skipped).
"""

import math

import numpy as np

import concourse.bass as bass
import concourse.mybir as mybir
from concourse import bacc
from concourse.tile import TileContext

B, H, S, D = 2, 16, 2048, 128
NCORES = 8
HPC = (B * H) // NCORES  # heads per core = 4
CHUNK = 512              # q-chunk width (1 PSUM bank of f32)
NCHUNK = S // CHUNK      # 4
KB = 128                 # k block
SCALE = 1.0 / math.sqrt(D)
F32 = mybir.dt.float32

_cache = {}


def _build_nc(reps=1):
    nc = bacc.Bacc()
    qT_d = nc.declare_dram_parameter("qT", [HPC, D, S], F32, isOutput=False)
    kT_d = nc.declare_dram_parameter("kT", [HPC, D, S], F32, isOutput=False)
    v_d = nc.declare_dram_parameter("v", [HPC, S, D], F32, isOutput=False)
    msk_d = nc.declare_dram_parameter("msk", [4, 128, CHUNK], F32, isOutput=False)
    oT_d = nc.declare_dram_parameter("oT", [HPC, D, S], F32, isOutput=True)

    with TileContext(nc) as tc:
        with (
            tc.tile_pool(name="const", bufs=1) as constp,
            tc.tile_pool(name="qkv", bufs=1) as qkvp,
            tc.tile_pool(name="exps", bufs=1) as expp,
            tc.tile_pool(name="work", bufs=1) as workp,
            tc.tile_pool(name="ps", bufs=1, space="PSUM") as psp,
        ):
            masks = constp.tile([128, 4, CHUNK], F32, name="masks")
            nc.sync.dma_start(out=masks, in_=msk_d.rearrange("j p q -> p j q"))
            ones_col = constp.tile([128, 1], F32, name="ones_col")
            nc.vector.memset(ones_col, 1.0)
            ones_row = constp.tile([1, 128], F32, name="ones_row")
            nc.vector.memset(ones_row, 1.0)

            for it in range(reps * HPC):
                hd = it % HPC
                qT = qkvp.tile([D, S], F32, tag="qT", bufs=2, name=f"qT{it}")
                nc.sync.dma_start(out=qT, in_=qT_d[hd])
                kT = qkvp.tile([D, S], F32, tag="kT", bufs=2, name=f"kT{it}")
                nc.sync.dma_start(out=kT, in_=kT_d[hd])
                vsb = qkvp.tile([128, S // 128, D], F32, tag="v", bufs=2,
                                name=f"v{it}")
                nc.sync.dma_start(
                    out=vsb, in_=v_d[hd].rearrange("(t p) d -> p t d", p=128)
                )

                for c in range(NCHUNK):
                    npair = 2 * c + 2  # pairs of 128-k-blocks (incl. 2 diag pairs)
                    qs = c * CHUNK
                    exp_tiles = []
                    for p in range(npair):
                        st = psp.tile([128, 2, CHUNK], F32, tag="sT", bufs=2,
                                      name=f"sT_{it}_{c}_{p}")
                        et = expp.tile([128, 2, CHUNK], F32, tag="expS", bufs=11,
                                       name=f"expS_{it}_{c}_{p}")
                        for i in range(2):
                            kb = 2 * p + i
                            nc.tensor.matmul(
                                st[:, i, :],
                                kT[:, kb * KB:(kb + 1) * KB],
                                qT[:, qs:qs + CHUNK],
                                start=True,
                                stop=True,
                            )
                        nc.scalar.activation(
                            et[:, :, :], st[:, :, :],
                            mybir.ActivationFunctionType.Exp,
                            scale=SCALE,
                        )
                        # last two pairs are the diagonal blocks j=0..3
                        if p >= npair - 2:
                            for i in range(2):
                                j = 2 * (p - (npair - 2)) + i
                                nc.gpsimd.tensor_tensor(
                                    out=et[:, i, :], in0=et[:, i, :],
                                    in1=masks[:, j, :], op=mybir.AluOpType.mult,
                                )
                        exp_tiles.append(et)

                    # denominator: tree-add over k blocks (DVE + GPSIMD in
                    # parallel), then reduce the remaining 128 partitions
                    # with a ones-matmul (PE).
                    accp = workp.tile([128, 2, CHUNK], F32, tag="accp", bufs=2,
                                      name=f"accp_{it}_{c}")
                    nc.vector.tensor_tensor(
                        out=accp, in0=exp_tiles[0], in1=exp_tiles[1],
                        op=mybir.AluOpType.add,
                    )
                    if npair >= 4:
                        half = npair // 2
                        for p in range(2, half):
                            nc.vector.tensor_tensor(
                                out=accp, in0=accp, in1=exp_tiles[p],
                                op=mybir.AluOpType.add,
                            )
                        accg = workp.tile([128, 2, CHUNK], F32, tag="accg",
                                          bufs=2, name=f"accg_{it}_{c}")
                        nc.gpsimd.tensor_tensor(
                            out=accg, in0=exp_tiles[half],
                            in1=exp_tiles[half + 1], op=mybir.AluOpType.add,
                        )
                        for p in range(half + 2, npair):
                            nc.gpsimd.tensor_tensor(
                                out=accg, in0=accg, in1=exp_tiles[p],
                                op=mybir.AluOpType.add,
                            )
                        nc.vector.tensor_tensor(
                            out=accp, in0=accp, in1=accg,
                            op=mybir.AluOpType.add,
                        )
                    acc = workp.tile([128, CHUNK], F32, tag="acc", bufs=2,
                                     name=f"acc_{it}_{c}")
                    nc.vector.tensor_tensor(
                        out=acc, in0=accp[:, 0, :], in1=accp[:, 1, :],
                        op=mybir.AluOpType.add,
                    )
                    l_ps = psp.tile([1, CHUNK], F32, tag="l", bufs=1,
                                    name=f"l_{it}_{c}")
                    nc.tensor.matmul(l_ps, ones_col, acc,
                                     start=True, stop=True)
                    rl = workp.tile([1, CHUNK], F32, tag="rl", bufs=2,
                                    name=f"rl_{it}_{c}")
                    nc.vector.reciprocal(rl, l_ps)
                    lbb = psp.tile([128, CHUNK], F32, tag="lbb", bufs=1,
                                   name=f"lbb_{it}_{c}")
                    nc.tensor.matmul(lbb, ones_row, rl, start=True, stop=True)
                    lb_sb = workp.tile([128, CHUNK], F32, tag="lb_sb", bufs=2,
                                       name=f"lbsb_{it}_{c}")
                    nc.vector.tensor_copy(out=lb_sb, in_=lbb)

                    # O^T accumulation over k blocks
                    oT = psp.tile([128, CHUNK], F32, tag="oT", bufs=2,
                                  name=f"oT_{it}_{c}")
                    nkb = 2 * npair
                    for kb in range(nkb):
                        nc.tensor.matmul(
                            oT,
                            vsb[:, kb, :],
                            exp_tiles[kb // 2][:, kb % 2, :],
                            start=(kb == 0),
                            stop=(kb == nkb - 1),
                        )
                    out_sb = workp.tile([128, CHUNK], F32, tag="out", bufs=2,
                                        name=f"out_{it}_{c}")
                    nc.vector.tensor_tensor(
                        out=out_sb, in0=oT, in1=lb_sb, op=mybir.AluOpType.mult,
                    )
                    nc.sync.dma_start(out=oT_d[hd][:, qs:qs + CHUNK], in_=out_sb)
    nc.compile()
    return nc


def _host_masks():
    # mask[j][k', q'] = 1 where q' >= k' + 128*j  (valid, keep)
    q = np.arange(CHUNK)[None, :]
    k = np.arange(128)[:, None]
    return np.stack(
        [(q >= k + 128 * j).astype(np.float32) for j in range(4)]
    )


def kernel(query, key, value):
    from concourse.bass_utils import run_bass_kernel_spmd

    if "nc" not in _cache:
        _cache["nc"] = _build_nc()
    nc = _cache["nc"]

    q = np.asarray(query, np.float32).reshape(B * H, S, D)
    k = np.asarray(key, np.float32).reshape(B * H, S, D)
    v = np.asarray(value, np.float32).reshape(B * H, S, D)
    msk = _host_masks()

    in_maps = []
    for c in range(NCORES):
        sl = slice(c * HPC, (c + 1) * HPC)
        in_maps.append({
            "qT": np.ascontiguousarray(q[sl].transpose(0, 2, 1)),
            "kT": np.ascontiguousarray(k[sl].transpose(0, 2, 1)),
            "v": np.ascontiguousarray(v[sl]),
            "msk": msk,
        })

    res = run_bass_kernel_spmd(nc, in_maps, list(range(NCORES))).results

    out = np.empty((B, S, H * D), np.float32)
    for c in range(NCORES):
        oT = res[c]["oT"]  # [HPC, D, S]
        for j in range(HPC):
            g = c * HPC + j
            b, h = g // H, g % H
            out[b, :, h * D:(h + 1) * D] = oT[j].T
    return out
